# revision 15
# baseline (speedup 1.0000x reference)
"""Trainium2 Bass kernel for nn_CopyMechanism.

Math (per batch b):
  out[g,c] = softmax_c(mask ? (score_h[g]+score_c[c]) : -inf)
             * sigmoid(gate_h[g]+gate_c[c]+b0)

The softmax over c of (score_h[g] + score_c[c]) equals softmax_c(score_c)
because score_h[g] is constant along c — copy_probs is independent of g and
w_attn[:H] drops out entirely. encoder_output is unused by the reference.
Scores are O(1) (unit-normal ctx, tiny weights), so exp needs no max
subtraction; masking is additive (-1e30 pre-exp -> exact 0).

v4 design — DMA-roofline oriented:
  * Host pre-transposes hidden/ctx to h-major bf16 and pre-swizzles so every
    DMA line is 4-8KB contiguous (128 descriptors/MB; v3's 1KB lines made
    HWDGE descriptor-gen a ~2us/MB serial cost on the sync queue).
  * Small weights ride the sync queue FIRST (two packed tensors), so the
    first ctx chunk's dots are never blocked on the slow SWDGE path.
  * Per 512-col ctx chunk (streamed, bufs=3): 8 PE matmuls with the [h,2]
    weight pair stationary accumulate rows [2,512] in PSUM; tiny [2,128]
    PE transposes put score/gate into column layout; masked exp with
    accum_out produces per-chunk Z partials; sigmoid tiles (scalar engine,
    bias=gate col, b0 folded into the gh row broadcast) all DURING the load.
  * Tail: Z partition-sum via K=128 matmul, p = e/Z, then out[c,g] =
    p[c]*sig tile muls split across vector+gpsimd, streamed to a swizzled
    bf16 HBM buffer; host unswizzles + upcasts.
"""
import sys

if "/opt/trn_rl_repo" not in sys.path:
    sys.path.insert(0, "/opt/trn_rl_repo")

import numpy as np
from contextlib import ExitStack

B, G, C, H = 8, 512, 4096, 1024
N_CORES = 8
P = 128
KH = H // P           # 8 h-blocks of 128
CJ = C // 512         # 8 c-chunks of 512
NCT = C // P          # 32 c-tiles of 128

_cache = {}


def _build():
    import concourse.bass as bass
    import concourse.tile as tile
    from concourse import bacc, mybir
    from concourse.masks import make_identity

    f32 = mybir.dt.float32
    bf16 = mybir.dt.bfloat16
    AF = mybir.ActivationFunctionType
    ALU = mybir.AluOpType

    nc = bacc.Bacc("TRN2", target_bir_lowering=False, debug=False,
                   num_devices=N_CORES)
    # wpack: [:, 0:16] = wc2 (sc/gc pairs per h-block), [:, 16:24] = wgh cols
    wp_d = nc.dram_tensor("wp", [P, 24], bf16, kind="ExternalInput").ap()
    # fpack: [:, 0:32] = additive mask bias (0 keep / -1e30 drop), [:, 32] = b0
    fp_d = nc.dram_tensor("fp", [P, 33], f32, kind="ExternalInput").ap()
    ht_d = nc.dram_tensor("ht", [P, KH, G], bf16, kind="ExternalInput").ap()
    ct_d = nc.dram_tensor("ct", [CJ, P, KH, 512], bf16,
                          kind="ExternalInput").ap()
    out_d = nc.dram_tensor("out", [CJ, P, 4, G], bf16,
                           kind="ExternalOutput").ap()

    with tile.TileContext(nc) as tc:
        with ExitStack() as ctx:
            sing = ctx.enter_context(tc.tile_pool(name="sing", bufs=1))
            hidp = ctx.enter_context(tc.tile_pool(name="hidp", bufs=1))
            ctp = ctx.enter_context(tc.tile_pool(name="ctp", bufs=4))
            rowp = ctx.enter_context(tc.tile_pool(name="rowp", bufs=2))
            sigp = ctx.enter_context(tc.tile_pool(name="sigp", bufs=8))
            outp = ctx.enter_context(tc.tile_pool(name="outp", bufs=3))
            smp = ctx.enter_context(tc.tile_pool(name="smp", bufs=1))
            dt_ps = ctx.enter_context(
                tc.tile_pool(name="dt_ps", bufs=2, space="PSUM"))
            tp_ps = ctx.enter_context(
                tc.tile_pool(name="tp_ps", bufs=2, space="PSUM"))
            wm_ps = ctx.enter_context(
                tc.tile_pool(name="wm_ps", bufs=1, space="PSUM"))

            # ---- all input DMAs on sync (HWDGE), tiny weights first ----
            wp = sing.tile([P, 24], bf16)
            nc.sync.dma_start(out=wp, in_=wp_d)
            fp = sing.tile([P, 33], f32)
            nc.sync.dma_start(out=fp, in_=fp_d)
            hid = hidp.tile([P, KH, G], bf16)
            nc.sync.dma_start(out=hid, in_=ht_d)
            cts = []
            for j in range(CJ):
                ctt = ctp.tile([P, KH, 512], bf16, tag="ct")
                nc.sync.dma_start(out=ctt, in_=ct_d[j])
                cts.append(ctt)
            wc2 = wp[:, 0:16]
            wgh = wp[:, 16:24]
            mb = fp[:, 0:32]

            ident = sing.tile([P, P], f32)
            make_identity(nc, ident)
            ones = sing.tile([P, 1], f32)
            nc.vector.memset(ones, 1.0)

            # PE clock warm-up: ~3.5us of junk matmuls during the DMA
            # preamble so the free-running activity window hits 2.4 GHz
            # before the first real dots arrive.
            warm_in = sing.tile([P, 64], f32)
            nc.vector.memset(warm_in, 0.0)
            warm = wm_ps.tile([64, 64], f32)
            for _ in range(56):
                nc.tensor.matmul(warm, warm_in, warm_in,
                                 start=True, stop=True)

            # ---- gh row = hid.T @ wgh + b0, broadcast to all partitions ----
            gh_ps = dt_ps.tile([1, G], f32, tag="dots")
            for k in range(KH):
                nc.tensor.matmul(gh_ps, wgh[:, k:k + 1], hid[:, k, :],
                                 start=(k == 0), stop=(k == KH - 1))
            gh_sb = sing.tile([1, G], f32)
            nc.scalar.activation(gh_sb, gh_ps, AF.Identity,
                                 bias=fp[0:1, 32:33])
            ghb = sing.tile([P, G], f32)
            nc.gpsimd.partition_broadcast(ghb, gh_sb)

            # interleaved (sc, gc) column pairs: sgcols[:, 2t] = sc col t,
            # sgcols[:, 2t+1] = gc col t
            sgcols = smp.tile([P, 2 * NCT], f32)
            msc = smp.tile([P, NCT], f32)     # masked scores (cols)
            e = smp.tile([P, NCT], f32)       # masked exp (cols)
            sigs = []                          # per-chunk tiles: no false dep

            for j in range(CJ):
                ctt = cts[j]
                dots = dt_ps.tile([2, 512], f32, tag="dots")
                for k in range(KH):
                    nc.tensor.matmul(dots, wc2[:, 2 * k:2 * k + 2],
                                     ctt[:, k, :],
                                     start=(k == 0), stop=(k == KH - 1))
                scgc = rowp.tile([2, 512], f32, tag="scgc")
                nc.vector.tensor_copy(scgc, dots)
                tp = tp_ps.tile([P, 8], f32, tag="tp")
                for i in range(4):
                    nc.tensor.transpose(tp[:, 2 * i:2 * i + 2],
                                        scgc[:, i * P:(i + 1) * P],
                                        ident[0:2, 0:2])
                nc.scalar.copy(sgcols[:, 8 * j:8 * j + 8], tp)
                sig_j = sigp.tile([P, 4, G], f32, tag="sig")
                sigs.append(sig_j)
                for i in range(4):
                    t = 4 * j + i
                    nc.scalar.activation(sig_j[:, i, :], ghb, AF.Sigmoid,
                                         bias=sgcols[:, 2 * t + 1:2 * t + 2])

            # ---- masked exp (one op: no ACT table thrash), Z, p = e/Z ----
            nc.vector.tensor_add(msc, sgcols[:, 0::2], mb)
            red = smp.tile([P, 1], f32)
            nc.scalar.activation(e, msc, AF.Exp, accum_out=red)
            z_ps = tp_ps.tile([1, 1], f32, tag="tp")
            nc.tensor.matmul(z_ps, red, ones, start=True, stop=True)
            z_sb = smp.tile([1, 1], f32)
            nc.scalar.copy(z_sb, z_ps)
            rz = smp.tile([1, 1], f32)
            nc.vector.reciprocal(rz, z_sb)
            rzb = smp.tile([P, 1], f32)
            nc.gpsimd.partition_broadcast(rzb, rz)
            pcc = smp.tile([P, NCT], f32)
            nc.vector.tensor_scalar(out=pcc, in0=e, scalar1=rzb[:, 0:1],
                                    scalar2=None, op0=ALU.mult)

            # ---- out[c,g] = p[c] * sig[c,g]; chunks 6-7 on the scalar
            # engine (activation Copy with per-partition scale), rest on
            # vector — single-writer tiles, overlapped with the stores ----
            for j in range(CJ):
                out4 = outp.tile([P, 4, G], bf16, tag="out4")
                for i in range(4):
                    t = 4 * j + i
                    if j >= 6:
                        nc.scalar.mul(out4[:, i, :], sigs[j][:, i, :],
                                      pcc[:, t:t + 1])
                    else:
                        nc.vector.tensor_scalar(
                            out=out4[:, i, :], in0=sigs[j][:, i, :],
                            scalar1=pcc[:, t:t + 1], scalar2=None,
                            op0=ALU.mult)
                nc.sync.dma_start(out=out_d[j], in_=out4)

    nc.compile()
    return nc


def _get_nc():
    if "nc" not in _cache:
        _cache["nc"] = _build()
    return _cache["nc"]


def make_host_inputs(hidden_states, context_hidden, w_attn, w_gate, b_gate,
                     copy_mask):
    """Per-core input dicts: swizzled/bf16 tensors + repacked weights."""
    import ml_dtypes

    bf16 = ml_dtypes.bfloat16
    wa = np.asarray(w_attn, np.float32)
    wg = np.asarray(w_gate, np.float32)
    wp = np.zeros((P, 24), dtype=np.float32)
    # wc2[p, 2k+0] = w_attn[H + k*128 + p] (sc), wc2[p, 2k+1] = w_gate[H+...]
    wp[:, 0:16] = np.stack([wa[H:].reshape(KH, P), wg[H:].reshape(KH, P)],
                           axis=2).transpose(1, 0, 2).reshape(P, 2 * KH)
    wp[:, 16:24] = wg[:H].reshape(KH, P).T
    wp = np.ascontiguousarray(wp, dtype=bf16)
    bg = float(np.asarray(b_gate, np.float32).reshape(-1)[0])
    in_maps = []
    for b in range(B):
        fpk = np.zeros((P, 33), dtype=np.float32)
        # additive mask: 0 keep, -1e30 drop (exp -> exact 0)
        m = np.asarray(copy_mask[b], np.float32).reshape(NCT, P).T
        fpk[:, 0:32] = (m - 1.0) * 1e30
        fpk[0, 32] = bg
        ht = np.asarray(hidden_states[b], np.float32).T.astype(bf16)
        ht = np.ascontiguousarray(ht.reshape(KH, P, G).transpose(1, 0, 2))
        ct = np.asarray(context_hidden[b], np.float32).T.astype(bf16)
        # [H, C] -> [CJ, P, KH, 512]: ct[k*128+p, j*512+c] -> ctsw[j, p, k, c]
        ct = np.ascontiguousarray(
            ct.reshape(KH, P, CJ, 512).transpose(2, 1, 0, 3))
        in_maps.append({"ht": ht, "ct": ct, "wp": wp, "fp": fpk})
    return in_maps


def unswizzle_out(res_out):
    """[CJ, P, 4, G] bf16 -> [G, C] f32 (out[g, c], c = j*512 + i*128 + p)."""
    o = np.asarray(res_out).astype(np.float32)
    return o.transpose(0, 2, 1, 3).reshape(C, G).T


def kernel(hidden_states, context_hidden, encoder_output, w_attn, w_gate,
           b_gate, copy_mask):
    from concourse.bass_utils import run_bass_kernel_spmd

    nc = _get_nc()
    in_maps = make_host_inputs(hidden_states, context_hidden, w_attn, w_gate,
                               b_gate, copy_mask)
    res = run_bass_kernel_spmd(nc, in_maps, core_ids=list(range(N_CORES)))
    return np.stack([unswizzle_out(res.results[b]["out"]) for b in range(B)],
                    axis=0)
